# revision 15
# baseline (speedup 1.0000x reference)
"""Trainium2 Bass kernel: DLRM-style dot interaction.

Computes, per sample b: G = X_b @ X_b^T for X_b = concat_features[b] of
shape [K=64, D=128], then gathers the strictly-lower-triangular entries
(row-major) into out[b] of length K*(K-1)/2 = 2016.

Sharding: pure data parallel over the batch axis, 8192/8 = 1024 samples
per NeuronCore.

Per-core dataflow (v2 — sample-major I/O to keep DMA descriptors big):
  - input DMA: [128 samples on partitions, 8192 free] -> one contiguous
    32KB run per partition (instead of 64x 512B runs per sample)
  - k-row transposes: T1_k = transpose(xs[:, k*128:(k+1)*128]) gives
    [128 (d), 128 (s)]; 4 per PSUM bank, one DVE copy evacuates each
    quad into XT [128 (d), (k, s)]
  - per sample s: Gram matmul with lhsT = rhs = XT[:, :, s] (stride-128
    column gather); two samples run concurrently via col-tiling
    (tile_position (0,0)/(0,64)); 8 samples share one PSUM tile,
    evacuated by one ScalarE copy into staging
    [part = (i, parity), free = (pair g, l)]
  - per Gram row i: one ragged DMA per supertile writes the first i
    entries of row i for both parities straight into the packed output.
"""

import numpy as np
from contextlib import ExitStack

import bass_rust
import concourse.bass as bass
import concourse.tile as tile
from concourse import mybir
from concourse.masks import make_identity
from concourse.bass_utils import run_bass_kernel_spmd

B, K, D = 8192, 64, 128
N_CORES = 8
NPAIR = K * (K - 1) // 2  # 2016
F32 = mybir.dt.float32

TILE_S = 128  # samples per input tile
QUAD = 4      # k-rows per transpose PSUM bank


def build_program(n_samples, supertile=256, legalize=True, merge_output=True):
    """Build the single-core Bass program (SPMD across all cores)."""
    assert n_samples % supertile == 0 and supertile % TILE_S == 0
    half_pairs = supertile // 2  # pairs per supertile

    nc = bass.Bass()
    x = nc.declare_dram_parameter("x", [n_samples, K, D], F32, isOutput=False)
    out = nc.declare_dram_parameter("out", [n_samples, NPAIR], F32, isOutput=True)

    with ExitStack() as ctx:
        tc = ctx.enter_context(tile.TileContext(nc))
        singles = ctx.enter_context(tc.tile_pool(name="singles", bufs=1))
        xs_pool = ctx.enter_context(tc.tile_pool(name="xs", bufs=2))
        xt_pool = ctx.enter_context(tc.tile_pool(name="xt", bufs=2))
        tps_pool = ctx.enter_context(
            tc.tile_pool(name="tps", bufs=3, space=bass.MemorySpace.PSUM)
        )
        gps_pool = ctx.enter_context(
            tc.tile_pool(name="gps", bufs=3, space=bass.MemorySpace.PSUM)
        )
        stage_pool = ctx.enter_context(tc.tile_pool(name="stage", bufs=2))

        identity = singles.tile([128, 128], F32)
        make_identity(nc, identity[:])

        # Warmup: transpose the identity into a throwaway PSUM tile so the PE
        # observes the GpSimd (identity-build) semaphore here. Real transposes
        # then carry at most one sync wait each (walrus LDW wait-slot limit).
        dummy_pool = ctx.enter_context(
            tc.tile_pool(name="dummy", bufs=1, space=bass.MemorySpace.PSUM)
        )
        dummy = dummy_pool.tile([128, 128], F32)
        nc.tensor.transpose(dummy[:], identity[:], identity[:])

        n_quads = K // QUAD  # 16 transpose-quads per input tile
        grp_s = 8            # samples per gram PSUM tile
        n_grp = TILE_S // grp_s  # 16 gram groups per input tile

        def emit_gram_group(xt, stage, pair0, j):
            """8 samples (ls = 8j..8j+7) -> one gps tile -> stage copy."""
            gps = gps_pool.tile([128, grp_s // 2 * K], F32)
            for q in range(grp_s // 2):
                a = j * grp_s + 2 * q
                nc.tensor.matmul(
                    gps[0:K, q * K : (q + 1) * K],
                    lhsT=xt[:, a * K : (a + 1) * K],
                    rhs=xt[:, a * K : (a + 1) * K],
                    tile_position=(0, 0),
                )
                nc.tensor.matmul(
                    gps[K:128, q * K : (q + 1) * K],
                    lhsT=xt[:, (a + 1) * K : (a + 2) * K],
                    rhs=xt[:, (a + 1) * K : (a + 2) * K],
                    tile_position=(0, 64),
                )
            g0 = pair0 + j * (grp_s // 2)
            nc.scalar.copy(stage[:, g0 * K : (g0 + grp_s // 2) * K], gps[:])

        def emit_output(stage, t):
            # Ragged packed-triangle output DMAs for supertile t.
            stage3 = stage[:].rearrange("p (g l) -> p g l", l=K)
            out_view = out[t * supertile : (t + 1) * supertile].rearrange(
                "(g h) p -> h g p", h=2
            )
            for i in range(1, K):
                off = i * (i - 1) // 2
                if merge_output:
                    eng = nc.sync if i % 2 else nc.scalar
                    eng.dma_start(
                        out=out_view[:, :, off : off + i],
                        in_=stage3[i :: 64, :, 0:i],
                    )
                else:
                    for hf in range(2):
                        eng = nc.sync if (i + hf) % 2 else nc.scalar
                        eng.dma_start(
                            out=out_view[hf : hf + 1, :, off : off + i],
                            in_=stage3[i + 64 * hf : i + 64 * hf + 1, :, 0:i],
                        )

        n_tiles = n_samples // TILE_S
        tiles_per_super = supertile // TILE_S
        # (xt, stage, pair0, super_idx, last_of_super) of the tile whose gram
        # groups are deferred one tile so PE never stalls on the DVE copies.
        pending = None
        stage = None

        for idx in range(n_tiles):
            t, it = divmod(idx, tiles_per_super)
            if it == 0:
                stage = stage_pool.tile([128, half_pairs * K], F32)
            s0 = idx * TILE_S
            xs = xs_pool.tile([128, K * D], F32)
            nc.gpsimd.dma_start(
                out=xs[:, :],
                in_=x[s0 : s0 + TILE_S].rearrange("s k d -> s (k d)"),
            )
            xt = xt_pool.tile([128, K * TILE_S], F32)
            for qd in range(n_quads):
                tps = tps_pool.tile([128, QUAD * TILE_S], F32)
                for kq in range(QUAD):
                    k = qd * QUAD + kq
                    nc.tensor.transpose(
                        tps[:, kq * TILE_S : (kq + 1) * TILE_S],
                        xs[:, k * D : (k + 1) * D],
                        identity[:],
                    )
                xt4 = xt[:].rearrange("p (s k) -> p k s", k=K)
                nc.vector.tensor_copy(
                    xt4[:, qd * QUAD : (qd + 1) * QUAD, :],
                    tps[:].rearrange("p (q s) -> p q s", s=TILE_S),
                )
                if pending is not None and qd < n_grp:
                    emit_gram_group(*pending[:3], qd)
            if pending is not None:
                for j in range(n_quads, n_grp):
                    emit_gram_group(*pending[:3], j)
                if pending[4]:
                    emit_output(pending[1], pending[3])
            pending = (xt, stage, it * (TILE_S // 2), t, it == tiles_per_super - 1)

        for j in range(n_grp):
            emit_gram_group(*pending[:3], j)
        emit_output(pending[1], pending[3])
    return _split_multiwait(nc) if legalize else nc


def _split_multiwait(nc):
    """Legalize sync waits for the TPB ISA: each 64B engine instruction has a
    single wait slot, but Tile's scheduler sometimes attaches several waits
    (notably redundant same-engine slot-release waits). Hoist all but the last
    wait of any instruction onto same-engine NoOps inserted right before it —
    sound because engine queues dispatch strictly in order."""
    counter = [0]
    for f in nc.m.functions:
        for bb in f.blocks:
            il = bb.instructions
            if not any(
                ins.sync_info and ins.sync_info.on_wait
                and len(ins.sync_info.on_wait) > 1
                for ins in il
            ):
                continue
            new = []
            for ins in il:
                si = ins.sync_info
                waits = list(si.on_wait) if si and si.on_wait else []
                if len(waits) > 1:
                    for w in waits[:-1]:
                        counter[0] += 1
                        nop = bass_rust.InstNoOp(
                            name=f"I-sw{counter[0]}", ins=[], outs=[]
                        )
                        nop.engine = ins.engine
                        nop.sync_info = bass_rust.SyncInfo(
                            on_wait=[w], on_update=[]
                        )
                        new.append(nop)
                    ins.sync_info = bass_rust.SyncInfo(
                        on_wait=[waits[-1]], on_update=list(si.on_update or [])
                    )
                new.append(ins)
            bb.instructions = new
    return nc


_PROGRAM_CACHE = {}


def _get_program(n_samples):
    if n_samples not in _PROGRAM_CACHE:
        st = 256 if n_samples % 256 == 0 else n_samples
        _PROGRAM_CACHE[n_samples] = build_program(n_samples, supertile=st)
    return _PROGRAM_CACHE[n_samples]


def run_sharded(concat_features, **spmd_kwargs):
    """Shard over batch, run on all 8 cores, gather. Returns (out, results)."""
    xs = np.ascontiguousarray(np.asarray(concat_features), dtype=np.float32)
    n, k, d = xs.shape
    assert (k, d) == (K, D) and n % N_CORES == 0
    per = n // N_CORES
    nc = _get_program(per)
    in_maps = [{"x": xs[c * per : (c + 1) * per]} for c in range(N_CORES)]
    res = run_bass_kernel_spmd(nc, in_maps, list(range(N_CORES)), **spmd_kwargs)
    outs = [np.asarray(res.results[c]["out"]) for c in range(N_CORES)]
    return np.concatenate(outs, axis=0).astype(np.float32), res


def kernel(concat_features):
    out, _ = run_sharded(concat_features)
    return out


# revision 16
# speedup vs baseline: 1.6008x; 1.6008x over previous
"""Trainium2 Bass kernel: DLRM-style dot interaction.

Computes, per sample b: G = X_b @ X_b^T for X_b = concat_features[b] of
shape [K=64, D=128], then gathers the strictly-lower-triangular entries
(row-major) into out[b] of length K*(K-1)/2 = 2016.

Sharding: pure data parallel over the batch axis, 8192/8 = 1024 samples
per NeuronCore.

Per-core dataflow (v2 — sample-major I/O to keep DMA descriptors big):
  - input DMA: [128 samples on partitions, 8192 free] -> one contiguous
    32KB run per partition (instead of 64x 512B runs per sample)
  - k-row transposes: T1_k = transpose(xs[:, k*128:(k+1)*128]) gives
    [128 (d), 128 (s)]; 4 per PSUM bank, one DVE copy evacuates each
    quad into XT [128 (d), (k, s)]
  - per sample s: Gram matmul with lhsT = rhs = XT[:, :, s] (stride-128
    column gather); two samples run concurrently via col-tiling
    (tile_position (0,0)/(0,64)); 8 samples share one PSUM tile,
    evacuated by one ScalarE copy into staging
    [part = (i, parity), free = (pair g, l)]
  - per Gram row i: one ragged DMA per supertile writes the first i
    entries of row i for both parities straight into the packed output.
"""

import numpy as np
from contextlib import ExitStack

import bass_rust
import concourse.bass as bass
import concourse.tile as tile
from concourse import mybir
from concourse.masks import make_identity
from concourse.bass_utils import run_bass_kernel_spmd

B, K, D = 8192, 64, 128
N_CORES = 8
NPAIR = K * (K - 1) // 2  # 2016
F32 = mybir.dt.float32

TILE_S = 128  # samples per input tile
QUAD = 4      # k-rows per transpose PSUM bank


def build_program(n_samples, supertile=256, legalize=True, merge_output=False):
    """Build the single-core Bass program (SPMD across all cores)."""
    assert n_samples % supertile == 0 and supertile % TILE_S == 0
    half_pairs = supertile // 2  # pairs per supertile

    nc = bass.Bass()
    x = nc.declare_dram_parameter("x", [n_samples, K, D], F32, isOutput=False)
    out = nc.declare_dram_parameter("out", [n_samples, NPAIR], F32, isOutput=True)

    with ExitStack() as ctx:
        tc = ctx.enter_context(tile.TileContext(nc))
        singles = ctx.enter_context(tc.tile_pool(name="singles", bufs=1))
        xs_pool = ctx.enter_context(tc.tile_pool(name="xs", bufs=2))
        xt_pool = ctx.enter_context(tc.tile_pool(name="xt", bufs=2))
        tps_pool = ctx.enter_context(
            tc.tile_pool(name="tps", bufs=3, space=bass.MemorySpace.PSUM)
        )
        gps_pool = ctx.enter_context(
            tc.tile_pool(name="gps", bufs=3, space=bass.MemorySpace.PSUM)
        )
        stage_pool = ctx.enter_context(tc.tile_pool(name="stage", bufs=2))

        identity = singles.tile([128, 128], F32)
        make_identity(nc, identity[:])

        # Warmup: transpose the identity into a throwaway PSUM tile so the PE
        # observes the GpSimd (identity-build) semaphore here. Real transposes
        # then carry at most one sync wait each (walrus LDW wait-slot limit).
        dummy_pool = ctx.enter_context(
            tc.tile_pool(name="dummy", bufs=1, space=bass.MemorySpace.PSUM)
        )
        dummy = dummy_pool.tile([128, 128], F32)
        nc.tensor.transpose(dummy[:], identity[:], identity[:])

        n_quads = K // QUAD  # 16 transpose-quads per input tile
        grp_s = 8            # samples per gram PSUM tile
        n_grp = TILE_S // grp_s  # 16 gram groups per input tile

        def emit_gram_group(xt, stage, pair0, j):
            """8 samples (ls = 8j..8j+7) -> one gps tile -> stage copy."""
            gps = gps_pool.tile([128, grp_s // 2 * K], F32)
            for q in range(grp_s // 2):
                a = j * grp_s + 2 * q
                nc.tensor.matmul(
                    gps[0:K, q * K : (q + 1) * K],
                    lhsT=xt[:, a * K : (a + 1) * K],
                    rhs=xt[:, a * K : (a + 1) * K],
                    tile_position=(0, 0),
                )
                nc.tensor.matmul(
                    gps[K:128, q * K : (q + 1) * K],
                    lhsT=xt[:, (a + 1) * K : (a + 2) * K],
                    rhs=xt[:, (a + 1) * K : (a + 2) * K],
                    tile_position=(0, 64),
                )
            g0 = pair0 + j * (grp_s // 2)
            dst = stage[:, g0 * K : (g0 + grp_s // 2) * K]
            if j % 2:
                nc.vector.tensor_copy(dst, gps[:])
            else:
                nc.scalar.copy(dst, gps[:])

        def emit_output(stage, t):
            # Ragged packed-triangle output DMAs for supertile t.
            stage3 = stage[:].rearrange("p (g l) -> p g l", l=K)
            out_view = out[t * supertile : (t + 1) * supertile].rearrange(
                "(g h) p -> h g p", h=2
            )
            for i in range(1, K):
                off = i * (i - 1) // 2
                if merge_output:
                    eng = nc.sync if i % 2 else nc.scalar
                    eng.dma_start(
                        out=out_view[:, :, off : off + i],
                        in_=stage3[i :: 64, :, 0:i],
                    )
                else:
                    for hf in range(2):
                        eng = (nc.sync, nc.scalar, nc.gpsimd)[(2 * i + hf) % 3]
                        eng.dma_start(
                            out=out_view[hf : hf + 1, :, off : off + i],
                            in_=stage3[i + 64 * hf : i + 64 * hf + 1, :, 0:i],
                        )

        n_tiles = n_samples // TILE_S
        tiles_per_super = supertile // TILE_S
        # (xt, stage, pair0, super_idx, last_of_super) of the tile whose gram
        # groups are deferred one tile so PE never stalls on the DVE copies.
        pending = None
        stage = None

        for idx in range(n_tiles):
            t, it = divmod(idx, tiles_per_super)
            if it == 0:
                stage = stage_pool.tile([128, half_pairs * K], F32)
            s0 = idx * TILE_S
            xs = xs_pool.tile([128, K * D], F32)
            nc.gpsimd.dma_start(
                out=xs[:, :],
                in_=x[s0 : s0 + TILE_S].rearrange("s k d -> s (k d)"),
            )
            xt = xt_pool.tile([128, K * TILE_S], F32)
            for qd in range(n_quads):
                tps = tps_pool.tile([128, QUAD * TILE_S], F32)
                for kq in range(QUAD):
                    k = qd * QUAD + kq
                    nc.tensor.transpose(
                        tps[:, kq * TILE_S : (kq + 1) * TILE_S],
                        xs[:, k * D : (k + 1) * D],
                        identity[:],
                    )
                xt4 = xt[:].rearrange("p (s k) -> p k s", k=K)
                nc.vector.tensor_copy(
                    xt4[:, qd * QUAD : (qd + 1) * QUAD, :],
                    tps[:].rearrange("p (q s) -> p q s", s=TILE_S),
                )
                if pending is not None and qd < n_grp:
                    emit_gram_group(*pending[:3], qd)
            if pending is not None:
                for j in range(n_quads, n_grp):
                    emit_gram_group(*pending[:3], j)
                if pending[4]:
                    emit_output(pending[1], pending[3])
            pending = (xt, stage, it * (TILE_S // 2), t, it == tiles_per_super - 1)

        for j in range(n_grp):
            emit_gram_group(*pending[:3], j)
        emit_output(pending[1], pending[3])
    return _split_multiwait(nc) if legalize else nc


def _split_multiwait(nc):
    """Legalize sync waits for the TPB ISA: each 64B engine instruction has a
    single wait slot, but Tile's scheduler sometimes attaches several waits
    (notably redundant same-engine slot-release waits). Hoist all but the last
    wait of any instruction onto same-engine NoOps inserted right before it —
    sound because engine queues dispatch strictly in order."""
    counter = [0]
    for f in nc.m.functions:
        for bb in f.blocks:
            il = bb.instructions
            if not any(
                ins.sync_info and ins.sync_info.on_wait
                and len(ins.sync_info.on_wait) > 1
                for ins in il
            ):
                continue
            new = []
            for ins in il:
                si = ins.sync_info
                waits = list(si.on_wait) if si and si.on_wait else []
                if len(waits) > 1:
                    for w in waits[:-1]:
                        counter[0] += 1
                        nop = bass_rust.InstNoOp(
                            name=f"I-sw{counter[0]}", ins=[], outs=[]
                        )
                        nop.engine = ins.engine
                        nop.sync_info = bass_rust.SyncInfo(
                            on_wait=[w], on_update=[]
                        )
                        new.append(nop)
                    ins.sync_info = bass_rust.SyncInfo(
                        on_wait=[waits[-1]], on_update=list(si.on_update or [])
                    )
                new.append(ins)
            bb.instructions = new
    return nc


_PROGRAM_CACHE = {}


def _get_program(n_samples):
    if n_samples not in _PROGRAM_CACHE:
        st = 256 if n_samples % 256 == 0 else n_samples
        _PROGRAM_CACHE[n_samples] = build_program(n_samples, supertile=st)
    return _PROGRAM_CACHE[n_samples]


def run_sharded(concat_features, **spmd_kwargs):
    """Shard over batch, run on all 8 cores, gather. Returns (out, results)."""
    xs = np.ascontiguousarray(np.asarray(concat_features), dtype=np.float32)
    n, k, d = xs.shape
    assert (k, d) == (K, D) and n % N_CORES == 0
    per = n // N_CORES
    nc = _get_program(per)
    in_maps = [{"x": xs[c * per : (c + 1) * per]} for c in range(N_CORES)]
    res = run_bass_kernel_spmd(nc, in_maps, list(range(N_CORES)), **spmd_kwargs)
    outs = [np.asarray(res.results[c]["out"]) for c in range(N_CORES)]
    return np.concatenate(outs, axis=0).astype(np.float32), res


def kernel(concat_features):
    out, _ = run_sharded(concat_features)
    return out


# revision 17
# speedup vs baseline: 2.1321x; 1.3319x over previous
"""Trainium2 Bass kernel: DLRM-style dot interaction.

Computes, per sample b: G = X_b @ X_b^T for X_b = concat_features[b] of
shape [K=64, D=128], then gathers the strictly-lower-triangular entries
(row-major) into out[b] of length K*(K-1)/2 = 2016.

Sharding: pure data parallel over the batch axis, 8192/8 = 1024 samples
per NeuronCore.

Per-core dataflow (v2 — sample-major I/O to keep DMA descriptors big):
  - input DMA: [128 samples on partitions, 8192 free] -> one contiguous
    32KB run per partition (instead of 64x 512B runs per sample)
  - k-row transposes: T1_k = transpose(xs[:, k*128:(k+1)*128]) gives
    [128 (d), 128 (s)]; 4 per PSUM bank, one DVE copy evacuates each
    quad into XT [128 (d), (k, s)]
  - per sample s: Gram matmul with lhsT = rhs = XT[:, :, s] (stride-128
    column gather); two samples run concurrently via col-tiling
    (tile_position (0,0)/(0,64)); 8 samples share one PSUM tile,
    evacuated by one ScalarE copy into staging
    [part = (i, parity), free = (pair g, l)]
  - per Gram row i: one ragged DMA per supertile writes the first i
    entries of row i for both parities straight into the packed output.
"""

import numpy as np
from contextlib import ExitStack

import bass_rust
import concourse.bass as bass
import concourse.tile as tile
from concourse import mybir
from concourse.masks import make_identity
from concourse.bass_utils import run_bass_kernel_spmd

B, K, D = 8192, 64, 128
N_CORES = 8
NPAIR = K * (K - 1) // 2  # 2016
F32 = mybir.dt.float32

TILE_S = 128  # samples per input tile
QUAD = 4      # k-rows per transpose PSUM bank


def build_program(n_samples, supertile=256, legalize=True, merge_output=False):
    """Build the single-core Bass program (SPMD across all cores)."""
    assert n_samples % supertile == 0 and supertile % TILE_S == 0
    half_pairs = supertile // 2  # pairs per supertile

    nc = bass.Bass()
    x = nc.declare_dram_parameter("x", [n_samples, K, D], F32, isOutput=False)
    out = nc.declare_dram_parameter("out", [n_samples, NPAIR], F32, isOutput=True)

    with ExitStack() as ctx:
        tc = ctx.enter_context(tile.TileContext(nc))
        singles = ctx.enter_context(tc.tile_pool(name="singles", bufs=1))
        xs_pool = ctx.enter_context(tc.tile_pool(name="xs", bufs=2))
        xt_pool = ctx.enter_context(tc.tile_pool(name="xt", bufs=2))
        tps_pool = ctx.enter_context(
            tc.tile_pool(name="tps", bufs=3, space=bass.MemorySpace.PSUM)
        )
        gps_pool = ctx.enter_context(
            tc.tile_pool(name="gps", bufs=3, space=bass.MemorySpace.PSUM)
        )
        stage_pool = ctx.enter_context(tc.tile_pool(name="stage", bufs=2))

        identity = singles.tile([128, 128], F32)
        make_identity(nc, identity[:])

        # Warmup: transpose the identity into a throwaway PSUM tile so the PE
        # observes the GpSimd (identity-build) semaphore here. Real transposes
        # then carry at most one sync wait each (walrus LDW wait-slot limit).
        dummy_pool = ctx.enter_context(
            tc.tile_pool(name="dummy", bufs=1, space=bass.MemorySpace.PSUM)
        )
        dummy = dummy_pool.tile([128, 128], F32)
        nc.tensor.transpose(dummy[:], identity[:], identity[:])

        n_quads = K // QUAD  # 16 transpose-quads per input tile
        grp_s = 8            # samples per gram PSUM tile
        n_grp = TILE_S // grp_s  # 16 gram groups per input tile

        def emit_gram_group(xt, stage, pair0, j):
            """8 samples (ls = 8j..8j+7) -> one gps tile -> stage copy."""
            gps = gps_pool.tile([128, grp_s // 2 * K], F32)
            for q in range(grp_s // 2):
                a = j * grp_s + 2 * q
                nc.tensor.matmul(
                    gps[0:K, q * K : (q + 1) * K],
                    lhsT=xt[:, a * K : (a + 1) * K],
                    rhs=xt[:, a * K : (a + 1) * K],
                    tile_position=(0, 0),
                )
                nc.tensor.matmul(
                    gps[K:128, q * K : (q + 1) * K],
                    lhsT=xt[:, (a + 1) * K : (a + 2) * K],
                    rhs=xt[:, (a + 1) * K : (a + 2) * K],
                    tile_position=(0, 64),
                )
            g0 = pair0 + j * (grp_s // 2)
            nc.vector.tensor_copy(
                stage[:, g0 * K : (g0 + grp_s // 2) * K], gps[:]
            )

        def emit_output(stage, t):
            # Ragged packed-triangle output DMAs for supertile t.
            stage3 = stage[:].rearrange("p (g l) -> p g l", l=K)
            out_view = out[t * supertile : (t + 1) * supertile].rearrange(
                "(g h) p -> h g p", h=2
            )
            for i in range(1, K):
                off = i * (i - 1) // 2
                if merge_output:
                    eng = nc.sync if i % 2 else nc.scalar
                    eng.dma_start(
                        out=out_view[:, :, off : off + i],
                        in_=stage3[i :: 64, :, 0:i],
                    )
                else:
                    for hf in range(2):
                        seq = (2 * (i - 1) + hf) % 8
                        eng = (nc.sync, nc.scalar, nc.gpsimd, nc.scalar,
                               nc.gpsimd, nc.scalar, nc.gpsimd, nc.gpsimd)[seq]
                        eng.dma_start(
                            out=out_view[hf : hf + 1, :, off : off + i],
                            in_=stage3[i + 64 * hf : i + 64 * hf + 1, :, 0:i],
                        )

        n_tiles = n_samples // TILE_S
        tiles_per_super = supertile // TILE_S
        # (xt, stage, pair0, super_idx, last_of_super) of the tile whose gram
        # groups are deferred one tile so PE never stalls on the DVE copies.
        pending = None
        stage = None

        for idx in range(n_tiles):
            t, it = divmod(idx, tiles_per_super)
            if it == 0:
                stage = stage_pool.tile([128, half_pairs * K], F32)
            s0 = idx * TILE_S
            xs = xs_pool.tile([128, K * D], F32)
            nc.sync.dma_start(
                out=xs[:, :],
                in_=x[s0 : s0 + TILE_S].rearrange("s k d -> s (k d)"),
            )
            xt = xt_pool.tile([128, K * TILE_S], F32)
            for qd in range(n_quads):
                tps = tps_pool.tile([128, QUAD * TILE_S], F32)
                for kq in range(QUAD):
                    k = qd * QUAD + kq
                    nc.tensor.transpose(
                        tps[:, kq * TILE_S : (kq + 1) * TILE_S],
                        xs[:, k * D : (k + 1) * D],
                        identity[:],
                    )
                xt4 = xt[:].rearrange("p (s k) -> p k s", k=K)
                nc.vector.tensor_copy(
                    xt4[:, qd * QUAD : (qd + 1) * QUAD, :],
                    tps[:].rearrange("p (q s) -> p q s", s=TILE_S),
                )
                if pending is not None and qd < n_grp:
                    emit_gram_group(*pending[:3], qd)
            if pending is not None:
                for j in range(n_quads, n_grp):
                    emit_gram_group(*pending[:3], j)
                if pending[4]:
                    emit_output(pending[1], pending[3])
            pending = (xt, stage, it * (TILE_S // 2), t, it == tiles_per_super - 1)

        for j in range(n_grp):
            emit_gram_group(*pending[:3], j)
        emit_output(pending[1], pending[3])
    return _split_multiwait(nc) if legalize else nc


def _split_multiwait(nc):
    """Legalize sync waits for the TPB ISA: each 64B engine instruction has a
    single wait slot, but Tile's scheduler sometimes attaches several waits
    (notably redundant same-engine slot-release waits). Hoist all but the last
    wait of any instruction onto same-engine NoOps inserted right before it —
    sound because engine queues dispatch strictly in order."""
    counter = [0]
    for f in nc.m.functions:
        for bb in f.blocks:
            il = bb.instructions
            if not any(
                ins.sync_info and ins.sync_info.on_wait
                and len(ins.sync_info.on_wait) > 1
                for ins in il
            ):
                continue
            new = []
            for ins in il:
                si = ins.sync_info
                waits = list(si.on_wait) if si and si.on_wait else []
                if len(waits) > 1:
                    for w in waits[:-1]:
                        counter[0] += 1
                        nop = bass_rust.InstNoOp(
                            name=f"I-sw{counter[0]}", ins=[], outs=[]
                        )
                        nop.engine = ins.engine
                        nop.sync_info = bass_rust.SyncInfo(
                            on_wait=[w], on_update=[]
                        )
                        new.append(nop)
                    ins.sync_info = bass_rust.SyncInfo(
                        on_wait=[waits[-1]], on_update=list(si.on_update or [])
                    )
                new.append(ins)
            bb.instructions = new
    return nc


_PROGRAM_CACHE = {}


def _get_program(n_samples):
    if n_samples not in _PROGRAM_CACHE:
        st = 256 if n_samples % 256 == 0 else n_samples
        _PROGRAM_CACHE[n_samples] = build_program(n_samples, supertile=st)
    return _PROGRAM_CACHE[n_samples]


def run_sharded(concat_features, **spmd_kwargs):
    """Shard over batch, run on all 8 cores, gather. Returns (out, results)."""
    xs = np.ascontiguousarray(np.asarray(concat_features), dtype=np.float32)
    n, k, d = xs.shape
    assert (k, d) == (K, D) and n % N_CORES == 0
    per = n // N_CORES
    nc = _get_program(per)
    in_maps = [{"x": xs[c * per : (c + 1) * per]} for c in range(N_CORES)]
    res = run_bass_kernel_spmd(nc, in_maps, list(range(N_CORES)), **spmd_kwargs)
    outs = [np.asarray(res.results[c]["out"]) for c in range(N_CORES)]
    return np.concatenate(outs, axis=0).astype(np.float32), res


def kernel(concat_features):
    out, _ = run_sharded(concat_features)
    return out
